# revision 50
# baseline (speedup 1.0000x reference)
"""GAT message-passing layer (segment softmax + weighted scatter) on 8 trn2 cores.

Strategy: 1D-partition destination nodes across the 8 cores (1250 each); every
edge is routed to the core that owns its destination (the sharding hint's
"partition src_idx/dst_idx/messages" option), so cores run independently with
no collectives.

Host-side prep (index planning + data layout only): destinations are packed
into nw=48 windows of <=27 rows each (degree-balanced LPT), edges are slotted
into T tiles of 128 per window, and the per-edge message rows (source features
pre-scaled elementwise by w_src, bf16) are laid out in slot order so the
device reads them as a single contiguous stream -- no per-edge DMA descriptors
anywhere.  A per-slot one-hot over the window rows is also host-built.

Device-side per macro-chunk of 4 windows (software-pipelined at distance 2):
  - stream the message rows + one-hot,
  - per-edge logit s = row-sum of the pre-scaled row (2 bf16 tree-add levels
    at 2 elem/cyc on DVE + one 1x tensor_reduce),
  - t = s + s_dst (Pool engine, broadcast add); x = exp(leaky_relu(t)) as
    max(exp(t), 1 + 0.01t) (Activation engine exp + scaled copy, DVE max),
  - X = onehot * x (DVE, bf16 2x), then per tile a [128edge x 27dst] x
    [128edge x 128feat] PE matmul accumulates features and a second 1-column
    matmul accumulates the softmax denominator, both in PSUM,
  - on close: out = num * recip(den + empty_mask), then * mask/w_src[c]
    (un-scales the pre-scaled features) + h_type on isolated nodes.
"""

import math
import os
import sys

import numpy as np

for _p in ("/opt/trn_rl_repo", "/root/.axon_site/_ro/trn_rl_repo"):
    if os.path.isdir(_p) and _p not in sys.path:
        sys.path.insert(0, _p)

import ml_dtypes  # noqa: E402

import concourse.bacc as bacc  # noqa: E402
import concourse.bass as bass  # noqa: E402
import concourse.mybir as mybir  # noqa: E402
import concourse.tile as tile  # noqa: E402

F32 = mybir.dt.float32
BF16 = mybir.dt.bfloat16
BF = ml_dtypes.bfloat16

N_SENT = 100000
N_TYPE = 10000
D = 128
N_CORES = 8
LEAKY = 0.01

P = 128          # SBUF partitions (edge slots per tile)
W = 27           # destination rows per window (PSUM partition dim)
NW = 48          # windows per core
MC = 4           # windows per macro-chunk
NMC = NW // MC   # macro-chunks per core
WG = 4           # windows per feature-PSUM tile (full 512-f32 bank)


def _plan(src_idx, dst_idx, n_type=N_TYPE, n_cores=N_CORES):
    """Window assignment + edge slotting. Integer index work only."""
    dpc = n_type // n_cores
    deg = np.bincount(dst_idx, minlength=n_type)
    wof = np.empty(n_type, np.int64)
    rof = np.empty(n_type, np.int64)
    loads_all = np.zeros((n_cores, NW), np.int64)
    for c in range(n_cores):
        base = c * dpc
        counts = np.zeros(NW, np.int64)
        loads = np.zeros(NW, np.int64)
        for dl in np.argsort(-deg[base:base + dpc], kind="stable"):
            elig = np.where(counts < W, loads, np.iinfo(np.int64).max)
            w = int(np.argmin(elig))
            wof[base + dl] = w
            rof[base + dl] = counts[w]
            counts[w] += 1
            loads[w] += deg[base + dl]
        loads_all[c] = loads
    T = max(14, int(-(-loads_all.max() // P)))
    spw = T * P                       # slots per window
    nslots = NW * spw                 # per core

    # slot of each edge: edges grouped by (core, window), any order within
    dsti = dst_idx.astype(np.int64)
    core_of = dsti // dpc
    gkey = core_of * NW + wof[dsti]
    order = np.argsort(gkey, kind="stable")
    gcnt = np.bincount(gkey, minlength=n_cores * NW)
    gstart = np.zeros(n_cores * NW + 1, np.int64)
    gstart[1:] = np.cumsum(gcnt)
    slot = np.empty(len(order), np.int64)   # slot within the core, edge-order
    pos_in_g = np.arange(len(order)) - gstart[gkey[order]]
    slot[order] = (gkey[order] % NW) * spw + pos_in_g

    return {"dpc": dpc, "T": T, "deg": deg, "wof": wof, "rof": rof,
            "order": order, "slot": slot, "nslots": nslots}


def _in_maps(plan, h_sent, h_type, attn_w, src_idx, dst_idx):
    dpc, T, nslots = plan["dpc"], plan["T"], plan["nslots"]
    wof, rof, deg = plan["wof"], plan["rof"], plan["deg"]
    ntiles = NW * T
    w1 = attn_w[0, :D].astype(np.float32)
    w2 = attn_w[0, D:].astype(np.float32)
    assert np.abs(w1).min() > 1e-20
    hw16 = (h_sent * w1).astype(BF)            # pre-scaled message rows
    recw1 = (1.0 / w1).astype(np.float32)

    maps = []
    for c in range(N_CORES):
        base = c * dpc
        sel = plan["order"][(dst_idx[plan["order"]] // dpc) == c]
        slots = plan["slot"][sel]
        p_of = slots % P
        t_of = slots // P

        etab = np.zeros((P, ntiles * D), BF)
        etab_v = etab.reshape(P, ntiles, D)
        etab_v[p_of, t_of] = hw16[src_idx[sel]]
        # per-macro-chunk: [all tiles' cols 0:64 | all tiles' cols 64:128]
        TMD = NW // NMC * T
        etab = np.ascontiguousarray(
            etab.reshape(P, NMC, TMD, 2, 64).transpose(0, 1, 3, 2, 4)
        ).reshape(P, ntiles * D)
        oh = np.zeros((P, ntiles * W), BF)
        oh_v = oh.reshape(P, ntiles, W)
        oh_v[p_of, t_of, rof[dst_idx[sel]]] = 1.0

        # window-layout destination tables [W, NW*D]
        dl = np.arange(base, base + dpc)
        r_l, w_l = rof[dl], wof[dl]
        sdht = np.zeros((W, NW, D), np.float32)
        sdht[r_l, w_l] = h_type[dl]
        sdhtT = np.ascontiguousarray(
            sdht.transpose(2, 1, 0).reshape(D, NW * W)).astype(BF)
        mask = np.zeros((W, NW), np.float32)
        mask[r_l, w_l] = (deg[dl] > 0).astype(np.float32)
        imask = np.zeros((W, NW), np.float32)
        imask[r_l, w_l] = (deg[dl] == 0).astype(np.float32)
        imask[mask + imask == 0] = 1.0         # unused (w, r) slots
        htm = (sdht * imask[:, :, None]).astype(np.float32)
        mwc = (mask[:, :, None] * recw1[None, None, :]).astype(BF)
        w2rep = np.ascontiguousarray(
            np.broadcast_to(w2.astype(BF)[:, None], (D, P)))

        maps.append({
            "etab": etab, "oh": oh,
            "sdhtT": sdhtT,
            "w2rep": w2rep,
            "imask": np.ascontiguousarray(imask),
            "mwc": np.ascontiguousarray(mwc.reshape(W, NW * D)),
            "htm": np.ascontiguousarray(htm.reshape(W, NW * D)),
        })
    return maps


def _build(plan):
    T = plan["T"]
    ntiles = NW * T
    TM = MC * T                     # tiles per macro-chunk
    A = mybir.AluOpType

    nc = bacc.Bacc(None, target_bir_lowering=False, debug=False)
    etab_d = nc.dram_tensor("etab", [P, ntiles * D], BF16, kind="ExternalInput")
    oh_d = nc.dram_tensor("oh", [P, ntiles * W], BF16, kind="ExternalInput")
    sdht_d = nc.dram_tensor("sdhtT", [D, NW * W], BF16, kind="ExternalInput")
    w2_d = nc.dram_tensor("w2rep", [D, P], BF16, kind="ExternalInput")
    imask_d = nc.dram_tensor("imask", [W, NW], F32, kind="ExternalInput")
    mwc_d = nc.dram_tensor("mwc", [W, NW * D], BF16, kind="ExternalInput")
    htm_d = nc.dram_tensor("htm", [W, NW * D], F32, kind="ExternalInput")
    out_d = nc.dram_tensor("out_local", [W, NW * D], F32, kind="ExternalOutput")

    with tile.TileContext(nc) as tc:
        with (
            tc.tile_pool(name="const", bufs=1) as const,
            tc.tile_pool(name="work", bufs=2) as work,
            tc.tile_pool(name="hpool", bufs=5) as hpool,
            tc.tile_pool(name="opool", bufs=3) as opool,
            tc.tile_pool(name="psum", bufs=2, space="PSUM") as psum,
        ):
            # ---- consts ----
            sdht = const.tile([D, NW * W], BF16)
            w2t = const.tile([D, P], BF16)
            imask = const.tile([W, NW], F32)
            nc.sync.dma_start(out=imask[:], in_=imask_d[:, :])
            mwc = const.tile([W, NW * D], BF16)
            htm = const.tile([W, NW * D], F32)
            ones1 = const.tile([P, 1], BF16)
            nc.vector.memset(ones1[:], 1.0)

            sdrep = const.tile([P, NW * W], F32)
            numbuf = const.tile([W, NW * D], BF16)

            def sd_setup():
                # sdrep[p, w*W+r] = sum_c w2[c] * h_typeT[c, w*W+r]; the
                # column-replicated w2 lhsT replicates across partitions free
                nc.scalar.dma_start(out=sdht[:], in_=sdht_d[:, :])
                nc.scalar.dma_start(out=w2t[:], in_=w2_d[:, :])
                CH = 432
                for i in range(math.ceil(NW * W / CH)):
                    n = min(CH, NW * W - i * CH)
                    pt = psum.tile([P, CH], F32, tag="rep")
                    nc.tensor.matmul(out=pt[:, 0:n], lhsT=w2t[:],
                                     rhs=sdht[:, i * CH:i * CH + n],
                                     start=True, stop=True)
                    nc.vector.tensor_copy(out=sdrep[:, i * CH:i * CH + n],
                                          in_=pt[:, 0:n])

            # ---- main loop: software-pipelined (front of mc, back of mc-1) ----
            st = {}

            def front(mc):
                t0 = mc * TM
                hbuf = hpool.tile([P, TM * D], BF16, tag="hbuf", name="hbuf")
                qs = 1
                for q in range(qs):
                    a = TM * D * q // qs
                    b = TM * D * (q + 1) // qs
                    nc.sync.dma_start(out=hbuf[:, a:b],
                                      in_=etab_d[:, t0 * D + a: t0 * D + b])
                ohb = opool.tile([P, TM * W], BF16, tag="ohb", name="ohb")
                nc.scalar.dma_start(out=ohb[:],
                                    in_=oh_d[:, t0 * W:(t0 + TM) * W])

                # s = row-sum of pre-scaled rows (3 tree levels + reduce);
                # the stream stores [all tiles' lo-64 | all tiles' hi-64] so
                # this first add is flat and contiguous
                sL1 = work.tile([P, TM * 64], BF16, tag="sL1", name="sL1")
                nc.vector.tensor_tensor(out=sL1[:], in0=hbuf[:, 0:TM * 64],
                                        in1=hbuf[:, TM * 64:TM * D], op=A.add)
                sL14 = sL1[:].rearrange("p (t u f) -> p t u f", u=2, f=32)
                sL2 = work.tile([P, TM * 32], BF16, tag="sL2", name="sL2")
                sL23 = sL2[:].rearrange("p (t f) -> p t f", f=32)
                nc.vector.tensor_tensor(out=sL23, in0=sL14[:, :, 0, :],
                                        in1=sL14[:, :, 1, :], op=A.add)
                sL24 = sL2[:].rearrange("p (t u f) -> p t u f", u=2, f=16)
                sL3 = work.tile([P, TM * 16], BF16, tag="sL3", name="sL3")
                sL33 = sL3[:].rearrange("p (t f) -> p t f", f=16)
                nc.vector.tensor_tensor(out=sL33, in0=sL24[:, :, 0, :],
                                        in1=sL24[:, :, 1, :], op=A.add)
                sL34 = sL3[:].rearrange("p (t u f) -> p t u f", u=2, f=8)
                sL4 = work.tile([P, TM * 8], BF16, tag="sL4", name="sL4")
                sL43 = sL4[:].rearrange("p (t f) -> p t f", f=8)
                nc.vector.tensor_tensor(out=sL43, in0=sL34[:, :, 0, :],
                                        in1=sL34[:, :, 1, :], op=A.add)
                scol = work.tile([P, TM], F32, tag="scol", name="scol")
                nc.vector.tensor_reduce(out=scol[:], in_=sL43,
                                        axis=mybir.AxisListType.X, op=A.add)

                st[mc] = (hbuf, ohb, scol)

            def back(mc):
                hbuf, ohb, scol = st.pop(mc)
                HW_ = MC // 2                 # windows per half
                HT = HW_ * T                  # tiles per half
                X3 = ohb[:].rearrange("p (t r) -> p t r", r=W)
                hb3 = hbuf[:].rearrange("p (u t f) -> p t u f", u=2, f=64)
                fps, dps = [], None
                for h in range(2):
                    ts0 = h * HT
                    # t = s + s_dst  (Pool), exp / 1+0.01t, max, X for the half
                    tfull = work.tile([P, HT * W], F32, tag=f"tf{h}",
                                      name="tfull")
                    tfull4 = tfull[:].rearrange("p (w t r) -> p w t r",
                                                w=HW_, r=W)
                    scol4 = (scol[:, ts0:ts0 + HT]
                             .rearrange("p (w t) -> p w t", w=HW_)
                             .rearrange("p w (t a) -> p w t a", a=1)
                             .to_broadcast([P, HW_, T, W]))
                    sd0 = (mc * MC + h * HW_) * W
                    sdrep4 = (sdrep[:, sd0:sd0 + HW_ * W]
                              .rearrange("p (w r) -> p w r", r=W)
                              .rearrange("p w (a r) -> p w a r", a=1)
                              .to_broadcast([P, HW_, T, W]))
                    nc.gpsimd.tensor_tensor(out=tfull4, in0=scol4, in1=sdrep4,
                                            op=A.add)
                    # exp(leaky_relu(t)) == max(exp(t), exp(0.01t));
                    # exp(0.01t) ~= 1 + 0.01t on the branch where it wins
                    xfull = work.tile([P, HT * W], BF16, tag=f"xf{h}",
                                      name="xfull")
                    nc.scalar.activation(out=xfull[:], in_=tfull[:],
                                         func=mybir.ActivationFunctionType.Exp)
                    x01 = work.tile([P, HT * W], BF16, tag=f"x0{h}", name="x01")
                    nc.scalar.activation(out=x01[:], in_=tfull[:],
                                         func=mybir.ActivationFunctionType.Copy,
                                         scale=LEAKY, bias=1.0)
                    nc.vector.tensor_tensor(out=xfull[:], in0=xfull[:],
                                            in1=x01[:], op=A.max)
                    ohs = ohb[:, ts0 * W:(ts0 + HT) * W]
                    nc.vector.tensor_tensor(out=ohs, in0=ohs, in1=xfull[:],
                                            op=A.mult)
                    # PE scatter: features + denominator
                    if h == 0:
                        fpt = psum.tile([W, WG * D], F32, tag="fp", name="fpt")
                        fps.append(fpt)
                        dps = psum.tile([W, MC], F32, tag="dp")
                    fpt = fps[0]
                    for wl in range(h * HW_, (h + 1) * HW_):
                        c0 = (wl % WG) * D
                        for j in range(T):
                            t = wl * T + j
                            nc.tensor.matmul(out=fpt[:, c0:c0 + D],
                                             lhsT=X3[:, t, :],
                                             rhs=hb3[:, t, :, :],
                                             start=(j == 0), stop=(j == T - 1))
                            nc.tensor.matmul(out=dps[:, wl:wl + 1],
                                             lhsT=X3[:, t, :], rhs=ones1[:],
                                             start=(j == 0), stop=(j == T - 1))

                # close: num/den (+ guard for empty rows)
                w0 = mc * MC
                dadj = work.tile([W, MC], F32, tag="dadj", name="dadj")
                nc.vector.tensor_tensor(out=dadj[:], in0=dps[:],
                                        in1=imask[:, w0:w0 + MC], op=A.add)
                rec = work.tile([W, MC], F32, tag="rec", name="rec")
                nc.vector.reciprocal(out=rec[:], in_=dadj[:])
                nb = (numbuf[:, w0 * D:(w0 + MC) * D]
                      .rearrange("p (w f) -> p w f", f=D))
                rb = (rec[:].rearrange("p (w a) -> p w a", a=1)
                      .to_broadcast([W, MC, D]))
                fp3 = fps[0][:].rearrange("p (w f) -> p w f", f=D)
                nc.vector.tensor_tensor(out=nb, in0=fp3, in1=rb, op=A.mult)
                # final blend + un-scale for this chunk, then store
                a, b = w0 * D, (w0 + MC) * D
                nc.scalar.dma_start(out=mwc[:, a:b], in_=mwc_d[:, a:b])
                nc.scalar.dma_start(out=htm[:, a:b], in_=htm_d[:, a:b])
                nc.vector.tensor_tensor(out=numbuf[:, a:b], in0=numbuf[:, a:b],
                                        in1=mwc[:, a:b], op=A.mult)
                nc.vector.tensor_tensor(out=htm[:, a:b], in0=numbuf[:, a:b],
                                        in1=htm[:, a:b], op=A.add)
                nc.sync.dma_start(out=out_d[:, a:b], in_=htm[:, a:b])

            front(0)
            sd_setup()
            front(1)
            for mc in range(2, NMC):
                front(mc)
                back(mc - 2)
            back(NMC - 2)
            back(NMC - 1)


    nc.finalize()
    return nc


def prepare(h_sent, h_type, attn_w, src_idx, dst_idx):
    plan = _plan(np.asarray(src_idx), np.asarray(dst_idx))
    nc = _build(plan)
    maps = _in_maps(plan, np.asarray(h_sent, dtype=np.float32),
                    np.asarray(h_type, dtype=np.float32),
                    np.asarray(attn_w, dtype=np.float32),
                    np.asarray(src_idx), np.asarray(dst_idx))
    return plan, nc, maps


def unpermute(plan, results):
    dpc = plan["dpc"]
    out = np.empty((N_CORES * dpc, D), np.float32)
    for c in range(N_CORES):
        rows = results[c]["out_local"].reshape(W, NW, D)
        base = c * dpc
        dl = np.arange(base, base + dpc)
        out[base:base + dpc] = rows[plan["rof"][dl], plan["wof"][dl]]
    return out


def kernel(h_sent, h_type, attn_w, src_idx, dst_idx):
    from concourse.bass_utils import run_bass_kernel_spmd

    plan, nc, maps = prepare(h_sent, h_type, attn_w, src_idx, dst_idx)
    res = run_bass_kernel_spmd(nc, maps, list(range(N_CORES)))
    return unpermute(plan, res.results)


# revision 51
# speedup vs baseline: 1.0294x; 1.0294x over previous
"""GAT message-passing layer (segment softmax + weighted scatter) on 8 trn2 cores.

Strategy: 1D-partition destination nodes across the 8 cores (1250 each); every
edge is routed to the core that owns its destination (the sharding hint's
"partition src_idx/dst_idx/messages" option), so cores run independently with
no collectives.

Host-side prep (index planning + data layout only): destinations are packed
into nw=48 windows of <=27 rows each (degree-balanced LPT), edges are slotted
into T tiles of 128 per window, and the per-edge message rows (source features
pre-scaled elementwise by w_src, bf16) are laid out in slot order so the
device reads them as a single contiguous stream -- no per-edge DMA descriptors
anywhere.  A per-slot one-hot over the window rows is also host-built.

Device-side per macro-chunk of 4 windows (software-pipelined at distance 2):
  - stream the message rows + one-hot,
  - per-edge logit s = row-sum of the pre-scaled row (2 bf16 tree-add levels
    at 2 elem/cyc on DVE + one 1x tensor_reduce),
  - t = s + s_dst (Pool engine, broadcast add); x = exp(leaky_relu(t)) as
    max(exp(t), 1 + 0.01t) (Activation engine exp + scaled copy, DVE max),
  - X = onehot * x (DVE, bf16 2x), then per tile a [128edge x 27dst] x
    [128edge x 128feat] PE matmul accumulates features and a second 1-column
    matmul accumulates the softmax denominator, both in PSUM,
  - on close: out = num * recip(den + empty_mask), then * mask/w_src[c]
    (un-scales the pre-scaled features) + h_type on isolated nodes.
"""

import math
import os
import sys

import numpy as np

for _p in ("/opt/trn_rl_repo", "/root/.axon_site/_ro/trn_rl_repo"):
    if os.path.isdir(_p) and _p not in sys.path:
        sys.path.insert(0, _p)

import ml_dtypes  # noqa: E402

import concourse.bacc as bacc  # noqa: E402
import concourse.bass as bass  # noqa: E402
import concourse.mybir as mybir  # noqa: E402
import concourse.tile as tile  # noqa: E402

F32 = mybir.dt.float32
BF16 = mybir.dt.bfloat16
BF = ml_dtypes.bfloat16

N_SENT = 100000
N_TYPE = 10000
D = 128
N_CORES = 8
LEAKY = 0.01

P = 128          # SBUF partitions (edge slots per tile)
W = 27           # destination rows per window (PSUM partition dim)
NW = 48          # windows per core
MC = 4           # windows per macro-chunk
NMC = NW // MC   # macro-chunks per core
WG = 2           # windows per feature-PSUM tile


def _plan(src_idx, dst_idx, n_type=N_TYPE, n_cores=N_CORES):
    """Window assignment + edge slotting. Integer index work only."""
    dpc = n_type // n_cores
    deg = np.bincount(dst_idx, minlength=n_type)
    wof = np.empty(n_type, np.int64)
    rof = np.empty(n_type, np.int64)
    loads_all = np.zeros((n_cores, NW), np.int64)
    for c in range(n_cores):
        base = c * dpc
        counts = np.zeros(NW, np.int64)
        loads = np.zeros(NW, np.int64)
        for dl in np.argsort(-deg[base:base + dpc], kind="stable"):
            elig = np.where(counts < W, loads, np.iinfo(np.int64).max)
            w = int(np.argmin(elig))
            wof[base + dl] = w
            rof[base + dl] = counts[w]
            counts[w] += 1
            loads[w] += deg[base + dl]
        loads_all[c] = loads
    T = max(14, int(-(-loads_all.max() // P)))
    spw = T * P                       # slots per window
    nslots = NW * spw                 # per core

    # slot of each edge: edges grouped by (core, window), any order within
    dsti = dst_idx.astype(np.int64)
    core_of = dsti // dpc
    gkey = core_of * NW + wof[dsti]
    order = np.argsort(gkey, kind="stable")
    gcnt = np.bincount(gkey, minlength=n_cores * NW)
    gstart = np.zeros(n_cores * NW + 1, np.int64)
    gstart[1:] = np.cumsum(gcnt)
    slot = np.empty(len(order), np.int64)   # slot within the core, edge-order
    pos_in_g = np.arange(len(order)) - gstart[gkey[order]]
    slot[order] = (gkey[order] % NW) * spw + pos_in_g

    return {"dpc": dpc, "T": T, "deg": deg, "wof": wof, "rof": rof,
            "order": order, "slot": slot, "nslots": nslots}


def _in_maps(plan, h_sent, h_type, attn_w, src_idx, dst_idx):
    dpc, T, nslots = plan["dpc"], plan["T"], plan["nslots"]
    wof, rof, deg = plan["wof"], plan["rof"], plan["deg"]
    ntiles = NW * T
    w1 = attn_w[0, :D].astype(np.float32)
    w2 = attn_w[0, D:].astype(np.float32)
    assert np.abs(w1).min() > 1e-20
    hw16 = (h_sent * w1).astype(BF)            # pre-scaled message rows
    recw1 = (1.0 / w1).astype(np.float32)

    maps = []
    for c in range(N_CORES):
        base = c * dpc
        sel = plan["order"][(dst_idx[plan["order"]] // dpc) == c]
        slots = plan["slot"][sel]
        p_of = slots % P
        t_of = slots // P

        etab = np.zeros((P, ntiles * D), BF)
        etab_v = etab.reshape(P, ntiles, D)
        etab_v[p_of, t_of] = hw16[src_idx[sel]]
        # per-macro-chunk: [all tiles' cols 0:64 | all tiles' cols 64:128]
        TMD = NW // NMC * T
        etab = np.ascontiguousarray(
            etab.reshape(P, NMC, TMD, 2, 64).transpose(0, 1, 3, 2, 4)
        ).reshape(P, ntiles * D)
        oh = np.zeros((P, ntiles * W), BF)
        oh_v = oh.reshape(P, ntiles, W)
        oh_v[p_of, t_of, rof[dst_idx[sel]]] = 1.0

        # window-layout destination tables [W, NW*D]
        dl = np.arange(base, base + dpc)
        r_l, w_l = rof[dl], wof[dl]
        sdht = np.zeros((W, NW, D), np.float32)
        sdht[r_l, w_l] = h_type[dl]
        sdhtT = np.ascontiguousarray(
            sdht.transpose(2, 1, 0).reshape(D, NW * W)).astype(BF)
        mask = np.zeros((W, NW), np.float32)
        mask[r_l, w_l] = (deg[dl] > 0).astype(np.float32)
        imask = np.zeros((W, NW), np.float32)
        imask[r_l, w_l] = (deg[dl] == 0).astype(np.float32)
        imask[mask + imask == 0] = 1.0         # unused (w, r) slots
        htm = (sdht * imask[:, :, None]).astype(np.float32)
        mwc = (mask[:, :, None] * recw1[None, None, :]).astype(BF)
        w2rep = np.ascontiguousarray(
            np.broadcast_to(w2.astype(BF)[:, None], (D, P)))

        maps.append({
            "etab": etab, "oh": oh,
            "sdhtT": sdhtT,
            "w2rep": w2rep,
            "imask": np.ascontiguousarray(imask),
            "mwc": np.ascontiguousarray(mwc.reshape(W, NW * D)),
            "htm": np.ascontiguousarray(htm.reshape(W, NW * D)),
        })
    return maps


def _build(plan):
    T = plan["T"]
    ntiles = NW * T
    TM = MC * T                     # tiles per macro-chunk
    A = mybir.AluOpType

    nc = bacc.Bacc(None, target_bir_lowering=False, debug=False)
    etab_d = nc.dram_tensor("etab", [P, ntiles * D], BF16, kind="ExternalInput")
    oh_d = nc.dram_tensor("oh", [P, ntiles * W], BF16, kind="ExternalInput")
    sdht_d = nc.dram_tensor("sdhtT", [D, NW * W], BF16, kind="ExternalInput")
    w2_d = nc.dram_tensor("w2rep", [D, P], BF16, kind="ExternalInput")
    imask_d = nc.dram_tensor("imask", [W, NW], F32, kind="ExternalInput")
    mwc_d = nc.dram_tensor("mwc", [W, NW * D], BF16, kind="ExternalInput")
    htm_d = nc.dram_tensor("htm", [W, NW * D], F32, kind="ExternalInput")
    out_d = nc.dram_tensor("out_local", [W, NW * D], F32, kind="ExternalOutput")

    with tile.TileContext(nc) as tc:
        with (
            tc.tile_pool(name="const", bufs=1) as const,
            tc.tile_pool(name="work", bufs=2) as work,
            tc.tile_pool(name="hpool", bufs=5) as hpool,
            tc.tile_pool(name="opool", bufs=3) as opool,
            tc.tile_pool(name="psum", bufs=2, space="PSUM") as psum,
        ):
            # ---- consts ----
            sdht = const.tile([D, NW * W], BF16)
            w2t = const.tile([D, P], BF16)
            imask = const.tile([W, NW], F32)
            nc.sync.dma_start(out=imask[:], in_=imask_d[:, :])
            mwc = const.tile([W, NW * D], BF16)
            htm = const.tile([W, NW * D], F32)
            ones1 = const.tile([P, 1], BF16)
            nc.vector.memset(ones1[:], 1.0)

            sdrep = const.tile([P, NW * W], F32)
            numbuf = const.tile([W, NW * D], BF16)

            def sd_setup():
                # sdrep[p, w*W+r] = sum_c w2[c] * h_typeT[c, w*W+r]; the
                # column-replicated w2 lhsT replicates across partitions free
                nc.scalar.dma_start(out=sdht[:], in_=sdht_d[:, :])
                nc.scalar.dma_start(out=w2t[:], in_=w2_d[:, :])
                CH = 432
                for i in range(math.ceil(NW * W / CH)):
                    n = min(CH, NW * W - i * CH)
                    pt = psum.tile([P, CH], F32, tag="rep")
                    nc.tensor.matmul(out=pt[:, 0:n], lhsT=w2t[:],
                                     rhs=sdht[:, i * CH:i * CH + n],
                                     start=True, stop=True)
                    nc.vector.tensor_copy(out=sdrep[:, i * CH:i * CH + n],
                                          in_=pt[:, 0:n])

            # ---- main loop: software-pipelined (front of mc, back of mc-1) ----
            st = {}

            def front(mc):
                t0 = mc * TM
                hbuf = hpool.tile([P, TM * D], BF16, tag="hbuf", name="hbuf")
                qs = 1
                for q in range(qs):
                    a = TM * D * q // qs
                    b = TM * D * (q + 1) // qs
                    nc.sync.dma_start(out=hbuf[:, a:b],
                                      in_=etab_d[:, t0 * D + a: t0 * D + b])
                ohb = opool.tile([P, TM * W], BF16, tag="ohb", name="ohb")
                nc.scalar.dma_start(out=ohb[:],
                                    in_=oh_d[:, t0 * W:(t0 + TM) * W])

                # s = row-sum of pre-scaled rows (3 tree levels + reduce);
                # the stream stores [all tiles' lo-64 | all tiles' hi-64] so
                # this first add is flat and contiguous
                sL1 = work.tile([P, TM * 64], BF16, tag="sL1", name="sL1")
                nc.vector.tensor_tensor(out=sL1[:], in0=hbuf[:, 0:TM * 64],
                                        in1=hbuf[:, TM * 64:TM * D], op=A.add)
                sL14 = sL1[:].rearrange("p (t u f) -> p t u f", u=2, f=32)
                sL2 = work.tile([P, TM * 32], BF16, tag="sL2", name="sL2")
                sL23 = sL2[:].rearrange("p (t f) -> p t f", f=32)
                nc.vector.tensor_tensor(out=sL23, in0=sL14[:, :, 0, :],
                                        in1=sL14[:, :, 1, :], op=A.add)
                sL24 = sL2[:].rearrange("p (t u f) -> p t u f", u=2, f=16)
                sL3 = work.tile([P, TM * 16], BF16, tag="sL3", name="sL3")
                sL33 = sL3[:].rearrange("p (t f) -> p t f", f=16)
                nc.vector.tensor_tensor(out=sL33, in0=sL24[:, :, 0, :],
                                        in1=sL24[:, :, 1, :], op=A.add)
                scol = work.tile([P, TM], F32, tag="scol", name="scol")
                nc.vector.tensor_reduce(out=scol[:], in_=sL33,
                                        axis=mybir.AxisListType.X, op=A.add)

                st[mc] = (hbuf, ohb, scol)

            def back(mc):
                hbuf, ohb, scol = st.pop(mc)
                HW_ = MC // 2                 # windows per half
                HT = HW_ * T                  # tiles per half
                X3 = ohb[:].rearrange("p (t r) -> p t r", r=W)
                hb3 = hbuf[:].rearrange("p (u t f) -> p t u f", u=2, f=64)
                fps, dps = [], None
                for h in range(2):
                    ts0 = h * HT
                    # t = s + s_dst  (Pool), exp / 1+0.01t, max, X for the half
                    tfull = work.tile([P, HT * W], F32, tag=f"tf{h}",
                                      name="tfull")
                    tfull4 = tfull[:].rearrange("p (w t r) -> p w t r",
                                                w=HW_, r=W)
                    scol4 = (scol[:, ts0:ts0 + HT]
                             .rearrange("p (w t) -> p w t", w=HW_)
                             .rearrange("p w (t a) -> p w t a", a=1)
                             .to_broadcast([P, HW_, T, W]))
                    sd0 = (mc * MC + h * HW_) * W
                    sdrep4 = (sdrep[:, sd0:sd0 + HW_ * W]
                              .rearrange("p (w r) -> p w r", r=W)
                              .rearrange("p w (a r) -> p w a r", a=1)
                              .to_broadcast([P, HW_, T, W]))
                    nc.gpsimd.tensor_tensor(out=tfull4, in0=scol4, in1=sdrep4,
                                            op=A.add)
                    # exp(leaky_relu(t)) == max(exp(t), exp(0.01t));
                    # exp(0.01t) ~= 1 + 0.01t on the branch where it wins
                    xfull = work.tile([P, HT * W], BF16, tag=f"xf{h}",
                                      name="xfull")
                    nc.scalar.activation(out=xfull[:], in_=tfull[:],
                                         func=mybir.ActivationFunctionType.Exp)
                    x01 = work.tile([P, HT * W], BF16, tag=f"x0{h}", name="x01")
                    nc.scalar.activation(out=x01[:], in_=tfull[:],
                                         func=mybir.ActivationFunctionType.Copy,
                                         scale=LEAKY, bias=1.0)
                    nc.vector.tensor_tensor(out=xfull[:], in0=xfull[:],
                                            in1=x01[:], op=A.max)
                    ohs = ohb[:, ts0 * W:(ts0 + HT) * W]
                    nc.vector.tensor_tensor(out=ohs, in0=ohs, in1=xfull[:],
                                            op=A.mult)
                    # PE scatter: features + denominator
                    fpt = psum.tile([W, WG * D], F32, tag=f"fp{h}",
                                    name="fpt")
                    fps.append(fpt)
                    if h == 0:
                        dps = psum.tile([W, MC], F32, tag="dp")
                    for wl in range(h * HW_, (h + 1) * HW_):
                        c0 = (wl % WG) * D
                        for j in range(T):
                            t = wl * T + j
                            nc.tensor.matmul(out=fpt[:, c0:c0 + D],
                                             lhsT=X3[:, t, :],
                                             rhs=hb3[:, t, :, :],
                                             start=(j == 0), stop=(j == T - 1))
                            nc.tensor.matmul(out=dps[:, wl:wl + 1],
                                             lhsT=X3[:, t, :], rhs=ones1[:],
                                             start=(j == 0), stop=(j == T - 1))

                # close: num/den (+ guard for empty rows)
                w0 = mc * MC
                dadj = work.tile([W, MC], F32, tag="dadj", name="dadj")
                nc.vector.tensor_tensor(out=dadj[:], in0=dps[:],
                                        in1=imask[:, w0:w0 + MC], op=A.add)
                rec = work.tile([W, MC], F32, tag="rec", name="rec")
                nc.vector.reciprocal(out=rec[:], in_=dadj[:])
                for k in range(MC // WG):
                    nb = (numbuf[:, (w0 + k * WG) * D:(w0 + (k + 1) * WG) * D]
                          .rearrange("p (w f) -> p w f", f=D))
                    rb = (rec[:, k * WG:(k + 1) * WG]
                          .rearrange("p (w a) -> p w a", a=1)
                          .to_broadcast([W, WG, D]))
                    fp3 = fps[k][:].rearrange("p (w f) -> p w f", f=D)
                    nc.vector.tensor_tensor(out=nb, in0=fp3, in1=rb, op=A.mult)
                # final blend + un-scale for this chunk, then store
                a, b = w0 * D, (w0 + MC) * D
                nc.scalar.dma_start(out=mwc[:, a:b], in_=mwc_d[:, a:b])
                nc.scalar.dma_start(out=htm[:, a:b], in_=htm_d[:, a:b])
                nc.vector.tensor_tensor(out=numbuf[:, a:b], in0=numbuf[:, a:b],
                                        in1=mwc[:, a:b], op=A.mult)
                nc.vector.tensor_tensor(out=htm[:, a:b], in0=numbuf[:, a:b],
                                        in1=htm[:, a:b], op=A.add)
                nc.sync.dma_start(out=out_d[:, a:b], in_=htm[:, a:b])

            front(0)
            sd_setup()
            front(1)
            for mc in range(2, NMC):
                front(mc)
                back(mc - 2)
            back(NMC - 2)
            back(NMC - 1)


    nc.finalize()
    return nc


def prepare(h_sent, h_type, attn_w, src_idx, dst_idx):
    plan = _plan(np.asarray(src_idx), np.asarray(dst_idx))
    nc = _build(plan)
    maps = _in_maps(plan, np.asarray(h_sent, dtype=np.float32),
                    np.asarray(h_type, dtype=np.float32),
                    np.asarray(attn_w, dtype=np.float32),
                    np.asarray(src_idx), np.asarray(dst_idx))
    return plan, nc, maps


def unpermute(plan, results):
    dpc = plan["dpc"]
    out = np.empty((N_CORES * dpc, D), np.float32)
    for c in range(N_CORES):
        rows = results[c]["out_local"].reshape(W, NW, D)
        base = c * dpc
        dl = np.arange(base, base + dpc)
        out[base:base + dpc] = rows[plan["rof"][dl], plan["wof"][dl]]
    return out


def kernel(h_sent, h_type, attn_w, src_idx, dst_idx):
    from concourse.bass_utils import run_bass_kernel_spmd

    plan, nc, maps = prepare(h_sent, h_type, attn_w, src_idx, dst_idx)
    res = run_bass_kernel_spmd(nc, maps, list(range(N_CORES)))
    return unpermute(plan, res.results)


# revision 52
# speedup vs baseline: 1.0383x; 1.0087x over previous
"""GAT message-passing layer (segment softmax + weighted scatter) on 8 trn2 cores.

Strategy: 1D-partition destination nodes across the 8 cores (1250 each); every
edge is routed to the core that owns its destination (the sharding hint's
"partition src_idx/dst_idx/messages" option), so cores run independently with
no collectives.

Host-side prep (index planning + data layout only): destinations are packed
into nw=48 windows of <=27 rows each (degree-balanced LPT), edges are slotted
into T tiles of 128 per window, and the per-edge message rows (source features
pre-scaled elementwise by w_src, bf16) are laid out in slot order so the
device reads them as a single contiguous stream -- no per-edge DMA descriptors
anywhere.  A per-slot one-hot over the window rows is also host-built.

Device-side per macro-chunk of 4 windows (software-pipelined at distance 2):
  - stream the message rows + one-hot,
  - per-edge logit s = row-sum of the pre-scaled row (2 bf16 tree-add levels
    at 2 elem/cyc on DVE + one 1x tensor_reduce),
  - t = s + s_dst (Pool engine, broadcast add); x = exp(leaky_relu(t)) as
    max(exp(t), 1 + 0.01t) (Activation engine exp + scaled copy, DVE max),
  - X = onehot * x (DVE, bf16 2x), then per tile a [128edge x 27dst] x
    [128edge x 128feat] PE matmul accumulates features and a second 1-column
    matmul accumulates the softmax denominator, both in PSUM,
  - on close: out = num * recip(den + empty_mask), then * mask/w_src[c]
    (un-scales the pre-scaled features) + h_type on isolated nodes.
"""

import math
import os
import sys

import numpy as np

for _p in ("/opt/trn_rl_repo", "/root/.axon_site/_ro/trn_rl_repo"):
    if os.path.isdir(_p) and _p not in sys.path:
        sys.path.insert(0, _p)

import ml_dtypes  # noqa: E402

import concourse.bacc as bacc  # noqa: E402
import concourse.bass as bass  # noqa: E402
import concourse.mybir as mybir  # noqa: E402
import concourse.tile as tile  # noqa: E402

F32 = mybir.dt.float32
BF16 = mybir.dt.bfloat16
BF = ml_dtypes.bfloat16

N_SENT = 100000
N_TYPE = 10000
D = 128
N_CORES = 8
LEAKY = 0.01

P = 128          # SBUF partitions (edge slots per tile)
W = 27           # destination rows per window (PSUM partition dim)
NW = 48          # windows per core
MC = 4           # windows per macro-chunk
NMC = NW // MC   # macro-chunks per core
WG = 2           # windows per feature-PSUM tile


def _plan(src_idx, dst_idx, n_type=N_TYPE, n_cores=N_CORES):
    """Window assignment + edge slotting. Integer index work only."""
    dpc = n_type // n_cores
    deg = np.bincount(dst_idx, minlength=n_type)
    wof = np.empty(n_type, np.int64)
    rof = np.empty(n_type, np.int64)
    loads_all = np.zeros((n_cores, NW), np.int64)
    for c in range(n_cores):
        base = c * dpc
        counts = np.zeros(NW, np.int64)
        loads = np.zeros(NW, np.int64)
        for dl in np.argsort(-deg[base:base + dpc], kind="stable"):
            elig = np.where(counts < W, loads, np.iinfo(np.int64).max)
            w = int(np.argmin(elig))
            wof[base + dl] = w
            rof[base + dl] = counts[w]
            counts[w] += 1
            loads[w] += deg[base + dl]
        loads_all[c] = loads
    T = max(14, int(-(-loads_all.max() // P)))
    spw = T * P                       # slots per window
    nslots = NW * spw                 # per core

    # slot of each edge: edges grouped by (core, window), any order within
    dsti = dst_idx.astype(np.int64)
    core_of = dsti // dpc
    gkey = core_of * NW + wof[dsti]
    order = np.argsort(gkey, kind="stable")
    gcnt = np.bincount(gkey, minlength=n_cores * NW)
    gstart = np.zeros(n_cores * NW + 1, np.int64)
    gstart[1:] = np.cumsum(gcnt)
    slot = np.empty(len(order), np.int64)   # slot within the core, edge-order
    pos_in_g = np.arange(len(order)) - gstart[gkey[order]]
    slot[order] = (gkey[order] % NW) * spw + pos_in_g

    return {"dpc": dpc, "T": T, "deg": deg, "wof": wof, "rof": rof,
            "order": order, "slot": slot, "nslots": nslots}


def _in_maps(plan, h_sent, h_type, attn_w, src_idx, dst_idx):
    dpc, T, nslots = plan["dpc"], plan["T"], plan["nslots"]
    wof, rof, deg = plan["wof"], plan["rof"], plan["deg"]
    ntiles = NW * T
    w1 = attn_w[0, :D].astype(np.float32)
    w2 = attn_w[0, D:].astype(np.float32)
    assert np.abs(w1).min() > 1e-20
    hw16 = (h_sent * w1).astype(BF)            # pre-scaled message rows
    recw1 = (1.0 / w1).astype(np.float32)

    maps = []
    for c in range(N_CORES):
        base = c * dpc
        sel = plan["order"][(dst_idx[plan["order"]] // dpc) == c]
        slots = plan["slot"][sel]
        p_of = slots % P
        t_of = slots // P

        etab = np.zeros((P, ntiles * D), BF)
        etab_v = etab.reshape(P, ntiles, D)
        etab_v[p_of, t_of] = hw16[src_idx[sel]]
        # per-macro-chunk: [all tiles' cols 0:64 | all tiles' cols 64:128]
        TMD = NW // NMC * T
        etab = np.ascontiguousarray(
            etab.reshape(P, NMC, TMD, 2, 64).transpose(0, 1, 3, 2, 4)
        ).reshape(P, ntiles * D)
        oh = np.zeros((P, ntiles * W), BF)
        oh_v = oh.reshape(P, ntiles, W)
        oh_v[p_of, t_of, rof[dst_idx[sel]]] = 1.0

        # window-layout destination tables [W, NW*D]
        dl = np.arange(base, base + dpc)
        r_l, w_l = rof[dl], wof[dl]
        sdht = np.zeros((W, NW, D), np.float32)
        sdht[r_l, w_l] = h_type[dl]
        sdhtT = np.ascontiguousarray(
            sdht.transpose(2, 1, 0).reshape(D, NW * W)).astype(BF)
        mask = np.zeros((W, NW), np.float32)
        mask[r_l, w_l] = (deg[dl] > 0).astype(np.float32)
        imask = np.zeros((W, NW), np.float32)
        imask[r_l, w_l] = (deg[dl] == 0).astype(np.float32)
        imask[mask + imask == 0] = 1.0         # unused (w, r) slots
        htm = (sdht * imask[:, :, None]).astype(BF)
        mwc = (mask[:, :, None] * recw1[None, None, :]).astype(BF)
        w2rep = np.ascontiguousarray(
            np.broadcast_to(w2.astype(BF)[:, None], (D, P)))

        maps.append({
            "etab": etab, "oh": oh,
            "sdhtT": sdhtT,
            "w2rep": w2rep,
            "imask": np.ascontiguousarray(imask),
            "mwc": np.ascontiguousarray(mwc.reshape(W, NW * D)),
            "htm": np.ascontiguousarray(htm.reshape(W, NW * D)),
        })
    return maps


def _build(plan):
    T = plan["T"]
    ntiles = NW * T
    TM = MC * T                     # tiles per macro-chunk
    A = mybir.AluOpType

    nc = bacc.Bacc(None, target_bir_lowering=False, debug=False)
    etab_d = nc.dram_tensor("etab", [P, ntiles * D], BF16, kind="ExternalInput")
    oh_d = nc.dram_tensor("oh", [P, ntiles * W], BF16, kind="ExternalInput")
    sdht_d = nc.dram_tensor("sdhtT", [D, NW * W], BF16, kind="ExternalInput")
    w2_d = nc.dram_tensor("w2rep", [D, P], BF16, kind="ExternalInput")
    imask_d = nc.dram_tensor("imask", [W, NW], F32, kind="ExternalInput")
    mwc_d = nc.dram_tensor("mwc", [W, NW * D], BF16, kind="ExternalInput")
    htm_d = nc.dram_tensor("htm", [W, NW * D], BF16, kind="ExternalInput")
    out_d = nc.dram_tensor("out_local", [W, NW * D], BF16, kind="ExternalOutput")

    with tile.TileContext(nc) as tc:
        with (
            tc.tile_pool(name="const", bufs=1) as const,
            tc.tile_pool(name="work", bufs=2) as work,
            tc.tile_pool(name="hpool", bufs=5) as hpool,
            tc.tile_pool(name="opool", bufs=3) as opool,
            tc.tile_pool(name="psum", bufs=2, space="PSUM") as psum,
        ):
            # ---- consts ----
            sdht = const.tile([D, NW * W], BF16)
            w2t = const.tile([D, P], BF16)
            imask = const.tile([W, NW], F32)
            nc.sync.dma_start(out=imask[:], in_=imask_d[:, :])
            mwc = const.tile([W, NW * D], BF16)
            htm = const.tile([W, NW * D], BF16)
            ones1 = const.tile([P, 1], BF16)
            nc.vector.memset(ones1[:], 1.0)

            sdrep = const.tile([P, NW * W], F32)
            numbuf = const.tile([W, NW * D], BF16)

            def sd_setup():
                # sdrep[p, w*W+r] = sum_c w2[c] * h_typeT[c, w*W+r]; the
                # column-replicated w2 lhsT replicates across partitions free
                nc.scalar.dma_start(out=sdht[:], in_=sdht_d[:, :])
                nc.scalar.dma_start(out=w2t[:], in_=w2_d[:, :])
                CH = 432
                for i in range(math.ceil(NW * W / CH)):
                    n = min(CH, NW * W - i * CH)
                    pt = psum.tile([P, CH], F32, tag="rep")
                    nc.tensor.matmul(out=pt[:, 0:n], lhsT=w2t[:],
                                     rhs=sdht[:, i * CH:i * CH + n],
                                     start=True, stop=True)
                    nc.vector.tensor_copy(out=sdrep[:, i * CH:i * CH + n],
                                          in_=pt[:, 0:n])

            # ---- main loop: software-pipelined (front of mc, back of mc-1) ----
            st = {}

            def front(mc):
                t0 = mc * TM
                hbuf = hpool.tile([P, TM * D], BF16, tag="hbuf", name="hbuf")
                qs = 1
                for q in range(qs):
                    a = TM * D * q // qs
                    b = TM * D * (q + 1) // qs
                    nc.sync.dma_start(out=hbuf[:, a:b],
                                      in_=etab_d[:, t0 * D + a: t0 * D + b])
                ohb = opool.tile([P, TM * W], BF16, tag="ohb", name="ohb")
                nc.scalar.dma_start(out=ohb[:],
                                    in_=oh_d[:, t0 * W:(t0 + TM) * W])

                # s = row-sum of pre-scaled rows (3 tree levels + reduce);
                # the stream stores [all tiles' lo-64 | all tiles' hi-64] so
                # this first add is flat and contiguous
                sL1 = work.tile([P, TM * 64], BF16, tag="sL1", name="sL1")
                nc.vector.tensor_tensor(out=sL1[:], in0=hbuf[:, 0:TM * 64],
                                        in1=hbuf[:, TM * 64:TM * D], op=A.add)
                sL14 = sL1[:].rearrange("p (t u f) -> p t u f", u=2, f=32)
                sL2 = work.tile([P, TM * 32], BF16, tag="sL2", name="sL2")
                sL23 = sL2[:].rearrange("p (t f) -> p t f", f=32)
                nc.vector.tensor_tensor(out=sL23, in0=sL14[:, :, 0, :],
                                        in1=sL14[:, :, 1, :], op=A.add)
                sL24 = sL2[:].rearrange("p (t u f) -> p t u f", u=2, f=16)
                sL3 = work.tile([P, TM * 16], BF16, tag="sL3", name="sL3")
                sL33 = sL3[:].rearrange("p (t f) -> p t f", f=16)
                nc.vector.tensor_tensor(out=sL33, in0=sL24[:, :, 0, :],
                                        in1=sL24[:, :, 1, :], op=A.add)
                scol = work.tile([P, TM], F32, tag="scol", name="scol")
                nc.vector.tensor_reduce(out=scol[:], in_=sL33,
                                        axis=mybir.AxisListType.X, op=A.add)

                st[mc] = (hbuf, ohb, scol)

            def back(mc):
                hbuf, ohb, scol = st.pop(mc)
                HW_ = MC // 2                 # windows per half
                HT = HW_ * T                  # tiles per half
                X3 = ohb[:].rearrange("p (t r) -> p t r", r=W)
                hb3 = hbuf[:].rearrange("p (u t f) -> p t u f", u=2, f=64)
                fps, dps = [], None
                for h in range(2):
                    ts0 = h * HT
                    # t = s + s_dst  (Pool), exp / 1+0.01t, max, X for the half
                    tfull = work.tile([P, HT * W], F32, tag=f"tf{h}",
                                      name="tfull")
                    tfull4 = tfull[:].rearrange("p (w t r) -> p w t r",
                                                w=HW_, r=W)
                    scol4 = (scol[:, ts0:ts0 + HT]
                             .rearrange("p (w t) -> p w t", w=HW_)
                             .rearrange("p w (t a) -> p w t a", a=1)
                             .to_broadcast([P, HW_, T, W]))
                    sd0 = (mc * MC + h * HW_) * W
                    sdrep4 = (sdrep[:, sd0:sd0 + HW_ * W]
                              .rearrange("p (w r) -> p w r", r=W)
                              .rearrange("p w (a r) -> p w a r", a=1)
                              .to_broadcast([P, HW_, T, W]))
                    nc.gpsimd.tensor_tensor(out=tfull4, in0=scol4, in1=sdrep4,
                                            op=A.add)
                    # exp(leaky_relu(t)) == max(exp(t), exp(0.01t));
                    # exp(0.01t) ~= 1 + 0.01t on the branch where it wins
                    xfull = work.tile([P, HT * W], BF16, tag=f"xf{h}",
                                      name="xfull")
                    nc.scalar.activation(out=xfull[:], in_=tfull[:],
                                         func=mybir.ActivationFunctionType.Exp)
                    x01 = work.tile([P, HT * W], BF16, tag=f"x0{h}", name="x01")
                    nc.scalar.activation(out=x01[:], in_=tfull[:],
                                         func=mybir.ActivationFunctionType.Copy,
                                         scale=LEAKY, bias=1.0)
                    nc.vector.tensor_tensor(out=xfull[:], in0=xfull[:],
                                            in1=x01[:], op=A.max)
                    ohs = ohb[:, ts0 * W:(ts0 + HT) * W]
                    nc.vector.tensor_tensor(out=ohs, in0=ohs, in1=xfull[:],
                                            op=A.mult)
                    # PE scatter: features + denominator
                    fpt = psum.tile([W, WG * D], F32, tag=f"fp{h}",
                                    name="fpt")
                    fps.append(fpt)
                    if h == 0:
                        dps = psum.tile([W, MC], F32, tag="dp")
                    for wl in range(h * HW_, (h + 1) * HW_):
                        c0 = (wl % WG) * D
                        for j in range(T):
                            t = wl * T + j
                            nc.tensor.matmul(out=fpt[:, c0:c0 + D],
                                             lhsT=X3[:, t, :],
                                             rhs=hb3[:, t, :, :],
                                             start=(j == 0), stop=(j == T - 1))
                            nc.tensor.matmul(out=dps[:, wl:wl + 1],
                                             lhsT=X3[:, t, :], rhs=ones1[:],
                                             start=(j == 0), stop=(j == T - 1))

                # close: num/den (+ guard for empty rows)
                w0 = mc * MC
                dadj = work.tile([W, MC], F32, tag="dadj", name="dadj")
                nc.vector.tensor_tensor(out=dadj[:], in0=dps[:],
                                        in1=imask[:, w0:w0 + MC], op=A.add)
                rec = work.tile([W, MC], F32, tag="rec", name="rec")
                nc.vector.reciprocal(out=rec[:], in_=dadj[:])
                for k in range(MC // WG):
                    nb = (numbuf[:, (w0 + k * WG) * D:(w0 + (k + 1) * WG) * D]
                          .rearrange("p (w f) -> p w f", f=D))
                    rb = (rec[:, k * WG:(k + 1) * WG]
                          .rearrange("p (w a) -> p w a", a=1)
                          .to_broadcast([W, WG, D]))
                    fp3 = fps[k][:].rearrange("p (w f) -> p w f", f=D)
                    nc.vector.tensor_tensor(out=nb, in0=fp3, in1=rb, op=A.mult)
                # final blend + un-scale for this chunk, then store
                a, b = w0 * D, (w0 + MC) * D
                nc.scalar.dma_start(out=mwc[:, a:b], in_=mwc_d[:, a:b])
                nc.scalar.dma_start(out=htm[:, a:b], in_=htm_d[:, a:b])
                nc.vector.tensor_tensor(out=numbuf[:, a:b], in0=numbuf[:, a:b],
                                        in1=mwc[:, a:b], op=A.mult)
                nc.vector.tensor_tensor(out=htm[:, a:b], in0=numbuf[:, a:b],
                                        in1=htm[:, a:b], op=A.add)
                nc.sync.dma_start(out=out_d[:, a:b], in_=htm[:, a:b])

            front(0)
            sd_setup()
            front(1)
            for mc in range(2, NMC):
                front(mc)
                back(mc - 2)
            back(NMC - 2)
            back(NMC - 1)


    nc.finalize()
    return nc


def prepare(h_sent, h_type, attn_w, src_idx, dst_idx):
    plan = _plan(np.asarray(src_idx), np.asarray(dst_idx))
    nc = _build(plan)
    maps = _in_maps(plan, np.asarray(h_sent, dtype=np.float32),
                    np.asarray(h_type, dtype=np.float32),
                    np.asarray(attn_w, dtype=np.float32),
                    np.asarray(src_idx), np.asarray(dst_idx))
    return plan, nc, maps


def unpermute(plan, results):
    dpc = plan["dpc"]
    out = np.empty((N_CORES * dpc, D), np.float32)
    for c in range(N_CORES):
        rows = results[c]["out_local"].astype(np.float32).reshape(W, NW, D)
        base = c * dpc
        dl = np.arange(base, base + dpc)
        out[base:base + dpc] = rows[plan["rof"][dl], plan["wof"][dl]]
    return out


def kernel(h_sent, h_type, attn_w, src_idx, dst_idx):
    from concourse.bass_utils import run_bass_kernel_spmd

    plan, nc, maps = prepare(h_sent, h_type, attn_w, src_idx, dst_idx)
    res = run_bass_kernel_spmd(nc, maps, list(range(N_CORES)))
    return unpermute(plan, res.results)


# revision 53
# speedup vs baseline: 1.0545x; 1.0156x over previous
"""GAT message-passing layer (segment softmax + weighted scatter) on 8 trn2 cores.

Strategy: 1D-partition destination nodes across the 8 cores (1250 each); every
edge is routed to the core that owns its destination (the sharding hint's
"partition src_idx/dst_idx/messages" option), so cores run independently with
no collectives.

Host-side prep (index planning + data layout only): destinations are packed
into nw=48 windows of <=27 rows each (degree-balanced LPT), edges are slotted
into T tiles of 128 per window, and the per-edge message rows (source features
pre-scaled elementwise by w_src, bf16) are laid out in slot order so the
device reads them as a single contiguous stream -- no per-edge DMA descriptors
anywhere.  A per-slot one-hot over the window rows is also host-built.

Device-side per macro-chunk of 4 windows (software-pipelined at distance 2):
  - stream the message rows + one-hot,
  - per-edge logit s = row-sum of the pre-scaled row (2 bf16 tree-add levels
    at 2 elem/cyc on DVE + one 1x tensor_reduce),
  - t = s + s_dst (Pool engine, broadcast add); x = exp(leaky_relu(t)) as
    max(exp(t), 1 + 0.01t) (Activation engine exp + scaled copy, DVE max),
  - X = onehot * x (DVE, bf16 2x), then per tile a [128edge x 27dst] x
    [128edge x 128feat] PE matmul accumulates features and a second 1-column
    matmul accumulates the softmax denominator, both in PSUM,
  - on close: out = num * recip(den + empty_mask), then * mask/w_src[c]
    (un-scales the pre-scaled features) + h_type on isolated nodes.
"""

import math
import os
import sys

import numpy as np

for _p in ("/opt/trn_rl_repo", "/root/.axon_site/_ro/trn_rl_repo"):
    if os.path.isdir(_p) and _p not in sys.path:
        sys.path.insert(0, _p)

import ml_dtypes  # noqa: E402

import concourse.bacc as bacc  # noqa: E402
import concourse.bass as bass  # noqa: E402
import concourse.mybir as mybir  # noqa: E402
import concourse.tile as tile  # noqa: E402

F32 = mybir.dt.float32
BF16 = mybir.dt.bfloat16
BF = ml_dtypes.bfloat16

N_SENT = 100000
N_TYPE = 10000
D = 128
N_CORES = 8
LEAKY = 0.01

P = 128          # SBUF partitions (edge slots per tile)
W = 27           # destination rows per window (PSUM partition dim)
NW = 48          # windows per core
MC = 4           # windows per macro-chunk
NMC = NW // MC   # macro-chunks per core
WG = 2           # windows per feature-PSUM tile


def _plan(src_idx, dst_idx, n_type=N_TYPE, n_cores=N_CORES):
    """Window assignment + edge slotting. Integer index work only."""
    dpc = n_type // n_cores
    deg = np.bincount(dst_idx, minlength=n_type)
    wof = np.empty(n_type, np.int64)
    rof = np.empty(n_type, np.int64)
    loads_all = np.zeros((n_cores, NW), np.int64)
    for c in range(n_cores):
        base = c * dpc
        counts = np.zeros(NW, np.int64)
        loads = np.zeros(NW, np.int64)
        for dl in np.argsort(-deg[base:base + dpc], kind="stable"):
            elig = np.where(counts < W, loads, np.iinfo(np.int64).max)
            w = int(np.argmin(elig))
            wof[base + dl] = w
            rof[base + dl] = counts[w]
            counts[w] += 1
            loads[w] += deg[base + dl]
        loads_all[c] = loads
    T = max(14, int(-(-loads_all.max() // P)))
    spw = T * P                       # slots per window
    nslots = NW * spw                 # per core

    # slot of each edge: edges grouped by (core, window), any order within
    dsti = dst_idx.astype(np.int64)
    core_of = dsti // dpc
    gkey = core_of * NW + wof[dsti]
    order = np.argsort(gkey, kind="stable")
    gcnt = np.bincount(gkey, minlength=n_cores * NW)
    gstart = np.zeros(n_cores * NW + 1, np.int64)
    gstart[1:] = np.cumsum(gcnt)
    slot = np.empty(len(order), np.int64)   # slot within the core, edge-order
    pos_in_g = np.arange(len(order)) - gstart[gkey[order]]
    slot[order] = (gkey[order] % NW) * spw + pos_in_g

    return {"dpc": dpc, "T": T, "deg": deg, "wof": wof, "rof": rof,
            "order": order, "slot": slot, "nslots": nslots}


def _in_maps(plan, h_sent, h_type, attn_w, src_idx, dst_idx):
    dpc, T, nslots = plan["dpc"], plan["T"], plan["nslots"]
    wof, rof, deg = plan["wof"], plan["rof"], plan["deg"]
    ntiles = NW * T
    w1 = attn_w[0, :D].astype(np.float32)
    w2 = attn_w[0, D:].astype(np.float32)
    assert np.abs(w1).min() > 1e-20
    hw16 = (h_sent * w1).astype(BF)            # pre-scaled message rows
    recw1 = (1.0 / w1).astype(np.float32)

    maps = []
    for c in range(N_CORES):
        base = c * dpc
        sel = plan["order"][(dst_idx[plan["order"]] // dpc) == c]
        slots = plan["slot"][sel]
        p_of = slots % P
        t_of = slots // P

        etab = np.zeros((P, ntiles * D), BF)
        etab_v = etab.reshape(P, ntiles, D)
        etab_v[p_of, t_of] = hw16[src_idx[sel]]
        # per-macro-chunk: [all tiles' cols 0:64 | all tiles' cols 64:128]
        TMD = NW // NMC * T
        etab = np.ascontiguousarray(
            etab.reshape(P, NMC, TMD, 2, 64).transpose(0, 1, 3, 2, 4)
        ).reshape(P, ntiles * D)
        oh = np.zeros((P, ntiles * W), BF)
        oh_v = oh.reshape(P, ntiles, W)
        oh_v[p_of, t_of, rof[dst_idx[sel]]] = 1.0

        # window-layout destination tables [W, NW*D]
        dl = np.arange(base, base + dpc)
        r_l, w_l = rof[dl], wof[dl]
        sdht = np.zeros((W, NW, D), np.float32)
        sdht[r_l, w_l] = h_type[dl]
        sdhtT = np.ascontiguousarray(
            sdht.transpose(2, 1, 0).reshape(D, NW * W)).astype(BF)
        mask = np.zeros((W, NW), np.float32)
        mask[r_l, w_l] = (deg[dl] > 0).astype(np.float32)
        imask = np.zeros((W, NW), np.float32)
        imask[r_l, w_l] = (deg[dl] == 0).astype(np.float32)
        imask[mask + imask == 0] = 1.0         # unused (w, r) slots
        htm = (sdht * imask[:, :, None]).astype(BF)
        mwc = (mask[:, :, None] * recw1[None, None, :]).astype(BF)
        w2rep = np.ascontiguousarray(
            np.broadcast_to(w2.astype(BF)[:, None], (D, P)))

        maps.append({
            "etab": etab, "oh": oh,
            "sdhtT": sdhtT,
            "w2rep": w2rep,
            "imask": np.ascontiguousarray(imask),
            "mwc": np.ascontiguousarray(mwc.reshape(W, NW * D)),
            "htm": np.ascontiguousarray(htm.reshape(W, NW * D)),
        })
    return maps


def _build(plan):
    T = plan["T"]
    ntiles = NW * T
    TM = MC * T                     # tiles per macro-chunk
    A = mybir.AluOpType

    nc = bacc.Bacc(None, target_bir_lowering=False, debug=False)
    etab_d = nc.dram_tensor("etab", [P, ntiles * D], BF16, kind="ExternalInput")
    oh_d = nc.dram_tensor("oh", [P, ntiles * W], BF16, kind="ExternalInput")
    sdht_d = nc.dram_tensor("sdhtT", [D, NW * W], BF16, kind="ExternalInput")
    w2_d = nc.dram_tensor("w2rep", [D, P], BF16, kind="ExternalInput")
    imask_d = nc.dram_tensor("imask", [W, NW], F32, kind="ExternalInput")
    mwc_d = nc.dram_tensor("mwc", [W, NW * D], BF16, kind="ExternalInput")
    htm_d = nc.dram_tensor("htm", [W, NW * D], BF16, kind="ExternalInput")
    out_d = nc.dram_tensor("out_local", [W, NW * D], BF16, kind="ExternalOutput")

    with tile.TileContext(nc) as tc:
        with (
            tc.tile_pool(name="const", bufs=1) as const,
            tc.tile_pool(name="work", bufs=2) as work,
            tc.tile_pool(name="hpool", bufs=5) as hpool,
            tc.tile_pool(name="opool", bufs=3) as opool,
            tc.tile_pool(name="psum", bufs=2, space="PSUM") as psum,
        ):
            # ---- consts ----
            sdht = const.tile([D, NW * W], BF16)
            w2t = const.tile([D, P], BF16)
            imask = const.tile([W, NW], F32)
            nc.sync.dma_start(out=imask[:], in_=imask_d[:, :])
            mwc = const.tile([W, NW * D], BF16)
            htm = const.tile([W, NW * D], BF16)
            ones1 = const.tile([P, 1], BF16)
            nc.vector.memset(ones1[:], 1.0)

            sdrep = const.tile([P, NW * W], F32)
            numbuf = const.tile([W, NW * D], BF16)

            def sd_setup():
                # sdrep[p, w*W+r] = sum_c w2[c] * h_typeT[c, w*W+r]; the
                # column-replicated w2 lhsT replicates across partitions free
                nc.scalar.dma_start(out=sdht[:], in_=sdht_d[:, :])
                nc.scalar.dma_start(out=w2t[:], in_=w2_d[:, :])
                CH = 432
                for i in range(math.ceil(NW * W / CH)):
                    n = min(CH, NW * W - i * CH)
                    pt = psum.tile([P, CH], F32, tag="rep")
                    nc.tensor.matmul(out=pt[:, 0:n], lhsT=w2t[:],
                                     rhs=sdht[:, i * CH:i * CH + n],
                                     start=True, stop=True)
                    nc.vector.tensor_copy(out=sdrep[:, i * CH:i * CH + n],
                                          in_=pt[:, 0:n])

            # ---- main loop: software-pipelined (front of mc, back of mc-1) ----
            st = {}

            def front(mc):
                t0 = mc * TM
                hbuf = hpool.tile([P, TM * D], BF16, tag="hbuf", name="hbuf")
                qs = 1
                for q in range(qs):
                    a = TM * D * q // qs
                    b = TM * D * (q + 1) // qs
                    nc.sync.dma_start(out=hbuf[:, a:b],
                                      in_=etab_d[:, t0 * D + a: t0 * D + b])
                ohb = opool.tile([P, TM * W], BF16, tag="ohb", name="ohb")
                nc.scalar.dma_start(out=ohb[:],
                                    in_=oh_d[:, t0 * W:(t0 + TM) * W])

                # s = row-sum of pre-scaled rows (3 tree levels + reduce);
                # the stream stores [all tiles' lo-64 | all tiles' hi-64] so
                # this first add is flat and contiguous
                sL1 = work.tile([P, TM * 64], BF16, tag="sL1", name="sL1")
                nc.vector.tensor_tensor(out=sL1[:], in0=hbuf[:, 0:TM * 64],
                                        in1=hbuf[:, TM * 64:TM * D], op=A.add)
                sL14 = sL1[:].rearrange("p (t u f) -> p t u f", u=2, f=32)
                sL2 = work.tile([P, TM * 32], BF16, tag="sL2", name="sL2")
                sL23 = sL2[:].rearrange("p (t f) -> p t f", f=32)
                nc.vector.tensor_tensor(out=sL23, in0=sL14[:, :, 0, :],
                                        in1=sL14[:, :, 1, :], op=A.add)
                sL24 = sL2[:].rearrange("p (t u f) -> p t u f", u=2, f=16)
                sL3 = work.tile([P, TM * 16], BF16, tag="sL3", name="sL3")
                sL33 = sL3[:].rearrange("p (t f) -> p t f", f=16)
                nc.vector.tensor_tensor(out=sL33, in0=sL24[:, :, 0, :],
                                        in1=sL24[:, :, 1, :], op=A.add)
                scol = work.tile([P, TM], F32, tag="scol", name="scol")
                nc.vector.tensor_reduce(out=scol[:], in_=sL33,
                                        axis=mybir.AxisListType.X, op=A.add)

                st[mc] = (hbuf, ohb, scol)

            def back(mc):
                hbuf, ohb, scol = st.pop(mc)
                HW_ = MC // 2                 # windows per half
                HT = HW_ * T                  # tiles per half
                X3 = ohb[:].rearrange("p (t r) -> p t r", r=W)
                hb3 = hbuf[:].rearrange("p (u t f) -> p t u f", u=2, f=64)
                fps, dps = [], None
                for h in range(2):
                    ts0 = h * HT
                    # t = s + s_dst  (Pool), exp / 1+0.01t, max, X for the half
                    tfull = work.tile([P, HT * W], F32, tag=f"tf{h}",
                                      name="tfull")
                    tfull4 = tfull[:].rearrange("p (w t r) -> p w t r",
                                                w=HW_, r=W)
                    scol4 = (scol[:, ts0:ts0 + HT]
                             .rearrange("p (w t) -> p w t", w=HW_)
                             .rearrange("p w (t a) -> p w t a", a=1)
                             .to_broadcast([P, HW_, T, W]))
                    sd0 = (mc * MC + h * HW_) * W
                    sdrep4 = (sdrep[:, sd0:sd0 + HW_ * W]
                              .rearrange("p (w r) -> p w r", r=W)
                              .rearrange("p w (a r) -> p w a r", a=1)
                              .to_broadcast([P, HW_, T, W]))
                    nc.gpsimd.tensor_tensor(out=tfull4, in0=scol4, in1=sdrep4,
                                            op=A.add)
                    # exp(leaky_relu(t)) == max(exp(t), exp(0.01t));
                    # exp(0.01t) ~= 1 + 0.01t on the branch where it wins
                    xfull = work.tile([P, HT * W], BF16, tag=f"xf{h}",
                                      name="xfull")
                    nc.scalar.activation(out=xfull[:], in_=tfull[:],
                                         func=mybir.ActivationFunctionType.Exp)
                    x01 = work.tile([P, HT * W], BF16, tag=f"x0{h}", name="x01")
                    nc.scalar.activation(out=x01[:], in_=tfull[:],
                                         func=mybir.ActivationFunctionType.Copy,
                                         scale=LEAKY, bias=1.0)
                    nc.vector.tensor_tensor(out=xfull[:], in0=xfull[:],
                                            in1=x01[:], op=A.max)
                    ohs = ohb[:, ts0 * W:(ts0 + HT) * W]
                    nc.vector.tensor_tensor(out=ohs, in0=ohs, in1=xfull[:],
                                            op=A.mult)
                    # PE scatter: features + denominator
                    fpt = psum.tile([W, WG * D], F32, tag=f"fp{h}",
                                    name="fpt")
                    fps.append(fpt)
                    if h == 0:
                        dps = psum.tile([W, MC], F32, tag="dp")
                    for wl in range(h * HW_, (h + 1) * HW_):
                        c0 = (wl % WG) * D
                        for j in range(T):
                            t = wl * T + j
                            nc.tensor.matmul(out=fpt[:, c0:c0 + D],
                                             lhsT=X3[:, t, :],
                                             rhs=hb3[:, t, :, :],
                                             start=(j == 0), stop=(j == T - 1))
                            nc.tensor.matmul(out=dps[:, wl:wl + 1],
                                             lhsT=X3[:, t, :], rhs=ones1[:],
                                             start=(j == 0), stop=(j == T - 1))

                # close: num/den (+ guard for empty rows)
                w0 = mc * MC
                dadj = work.tile([W, MC], F32, tag="dadj", name="dadj")
                nc.vector.tensor_tensor(out=dadj[:], in0=dps[:],
                                        in1=imask[:, w0:w0 + MC], op=A.add)
                rec = work.tile([W, MC], F32, tag="rec", name="rec")
                nc.vector.reciprocal(out=rec[:], in_=dadj[:])
                for k in range(MC // WG):
                    nb = (numbuf[:, (w0 + k * WG) * D:(w0 + (k + 1) * WG) * D]
                          .rearrange("p (w f) -> p w f", f=D))
                    rb = (rec[:, k * WG:(k + 1) * WG]
                          .rearrange("p (w a) -> p w a", a=1)
                          .to_broadcast([W, WG, D]))
                    fp3 = fps[k][:].rearrange("p (w f) -> p w f", f=D)
                    nc.vector.tensor_tensor(out=nb, in0=fp3, in1=rb, op=A.mult)
                # final blend + un-scale for this chunk, then store
                a, b = w0 * D, (w0 + MC) * D
                nc.sync.dma_start(out=mwc[:, a:b], in_=mwc_d[:, a:b])
                nc.sync.dma_start(out=htm[:, a:b], in_=htm_d[:, a:b])
                nc.vector.tensor_tensor(out=numbuf[:, a:b], in0=numbuf[:, a:b],
                                        in1=mwc[:, a:b], op=A.mult)
                nc.vector.tensor_tensor(out=htm[:, a:b], in0=numbuf[:, a:b],
                                        in1=htm[:, a:b], op=A.add)
                nc.sync.dma_start(out=out_d[:, a:b], in_=htm[:, a:b])

            front(0)
            sd_setup()
            front(1)
            for mc in range(2, NMC):
                front(mc)
                back(mc - 2)
            back(NMC - 2)
            back(NMC - 1)


    nc.finalize()
    return nc


def prepare(h_sent, h_type, attn_w, src_idx, dst_idx):
    plan = _plan(np.asarray(src_idx), np.asarray(dst_idx))
    nc = _build(plan)
    maps = _in_maps(plan, np.asarray(h_sent, dtype=np.float32),
                    np.asarray(h_type, dtype=np.float32),
                    np.asarray(attn_w, dtype=np.float32),
                    np.asarray(src_idx), np.asarray(dst_idx))
    return plan, nc, maps


def unpermute(plan, results):
    dpc = plan["dpc"]
    out = np.empty((N_CORES * dpc, D), np.float32)
    for c in range(N_CORES):
        rows = results[c]["out_local"].astype(np.float32).reshape(W, NW, D)
        base = c * dpc
        dl = np.arange(base, base + dpc)
        out[base:base + dpc] = rows[plan["rof"][dl], plan["wof"][dl]]
    return out


def kernel(h_sent, h_type, attn_w, src_idx, dst_idx):
    from concourse.bass_utils import run_bass_kernel_spmd

    plan, nc, maps = prepare(h_sent, h_type, attn_w, src_idx, dst_idx)
    res = run_bass_kernel_spmd(nc, maps, list(range(N_CORES)))
    return unpermute(plan, res.results)
